# revision 4
# baseline (speedup 1.0000x reference)
"""AutoCorrelationLoss Trainium2 kernel (8-core SPMD, data-parallel over batch).

Math: for each row x (length L=8192), with com = L - 128 = 8064 = 128*63:
  ac[k] = mean(x0c * (Y_k - mean(Y_k)))  where x0c = x[:com] - mean(x[:com])
Since sum(x0c) = 0, the mean(Y_k) term vanishes:
  com * ac[k] = c[k] = sum_j x0c[j] * x[j+k]
Decompose j = 63*t + p (t<128, p<63) and let X2[t, f] = x[63t + f] (f<191),
W = X2[:, :63] - mean(x[:com]).  Then with H = W.T @ X2  ([63, 191]):
  c[k] = sum_{p<63} H[p, p+k]   (a skew sum, k = 0..128)
which a diagonal-stride DMA through a DRAM bounce turns into a plain
column sum (ones matmul).  r[k] = c[k]/c[0];
loss = mean_{b,k} |r_fake - r_real|.

Per core: 4 batch rows x {fake, real} = 8 row-tensors, interleaved as 4
groups (fake_i, real_i) so each group's de-skew bounce + normalize + |diff|
tail pipelines against later groups' matmuls.  All matmuls in bf16
(1 cycle/row vs fp32's 4); fp32 PSUM accumulate keeps the final scalar
well inside the 2e-2 gate (measured ~5e-5).
"""

import sys

sys.path.insert(0, "/opt/trn_rl_repo")

import numpy as np

import concourse.bacc as bacc
import concourse.bass as bass
import concourse.mybir as mybir
import concourse.tile as tile
from concourse.bass_utils import run_bass_kernel_spmd
from concourse.tile_rust import add_dep_helper

B, L = 32, 8192
NCOEF = 128            # lags 0..128 -> 129 values
NK = NCOEF + 1         # 129
COM = L - NCOEF        # 8064 = 128 * 63
CH = 63                # chunk width (free dim of weights / H partition dim)
NT = COM // CH         # 128 contraction chunks -> full partition dim
HALO = CH + NCOEF      # 191
N_CORES = 8
ROWS_PER_CORE = B // N_CORES      # 4 batch rows per core
RT = 2 * ROWS_PER_CORE            # 8 row-tensors: [f0 r0 f1 r1 f2 r2 f3 r3]
NG = ROWS_PER_CORE                # 4 (fake, real) groups
HRT = RT // 2                     # 4 rows per load half

FP32 = mybir.dt.float32
BF16 = mybir.dt.bfloat16


def build_program():
    nc = bacc.Bacc(
        "TRN2",
        target_bir_lowering=False,
        debug=False,
        num_devices=N_CORES,
    )

    xin = nc.dram_tensor("xin", (RT, L), FP32, kind="ExternalInput")
    out = nc.dram_tensor("out", (1, NG), FP32, kind="ExternalOutput")

    with tile.TileContext(nc) as tc:
        with (
            tc.tile_pool(name="persist", bufs=1) as persist,
            tc.tile_pool(name="hdp", bufs=1, space=bass.MemorySpace.DRAM) as hdp,
            tc.tile_pool(name="hps", bufs=3, space=bass.MemorySpace.PSUM) as hps,
            tc.tile_pool(name="bps", bufs=2, space=bass.MemorySpace.PSUM) as bps,
            tc.tile_pool(name="cps", bufs=2, space=bass.MemorySpace.PSUM) as cps,
        ):
            ones_bf = persist.tile([NT, NT], BF16)      # partition-bcast weights
            nc.vector.memset(ones_bf[:], 1.0)
            ones_col = persist.tile([CH, 1], BF16)      # column-sum weights
            nc.vector.memset(ones_col[:], 1.0)

            xall = persist.tile([NT, RT, HALO], FP32)   # halo'd input
            xbf = persist.tile([NT, RT, HALO], BF16)    # bf16 rhs
            rowsums = persist.tile([NT, RT], FP32)
            msc = persist.tile([NT, RT], BF16)          # per-chunk means
            wbig = persist.tile([NT, RT, CH], BF16)     # centered weights
            hall = persist.tile([CH, RT, HALO], BF16)   # H matrices (SBUF)
            rbig = persist.tile([CH, RT, NK], BF16)     # de-skewed diagonals
            rnorm = persist.tile([1, RT * NK], FP32)    # c / c0
            diffs = persist.tile([1, NG, NK], FP32)     # r_fake - r_real
            absum = persist.tile([1, NG], FP32)

            hd = hdp.tile([CH, RT, HALO], BF16)         # DRAM de-skew bounce

            # --- loads (two halves, parallel trigger queues) + mean chain
            for h in range(2):
                sl = slice(h * HRT, (h + 1) * HRT)
                eng = nc.sync if h == 0 else nc.scalar
                src = bass.AP(xin, h * HRT * L, [[CH, NT], [L, HRT], [1, HALO]])
                eng.dma_start(xall[:, sl, :], src)

                nc.vector.tensor_copy(xbf[:, sl, :], xall[:, sl, :])
                nc.vector.tensor_reduce(
                    rowsums[:, sl], xall[:, sl, 0:CH],
                    mybir.AxisListType.X, mybir.AluOpType.add,
                )
                nc.vector.tensor_scalar_mul(msc[:, sl], rowsums[:, sl],
                                            1.0 / COM)
                # broadcast sum of per-chunk means (= row mean) over partitions
                mb = bps.tile([NT, HRT], FP32, tag="mb")
                nc.tensor.matmul(mb[:], ones_bf[:], msc[:, sl],
                                 start=True, stop=True)
                nc.vector.tensor_tensor(
                    wbig[:, sl, :], xall[:, sl, 0:CH],
                    mb[:].unsqueeze(2).broadcast_to([NT, HRT, CH]),
                    mybir.AluOpType.subtract,
                )

            # --- per (fake, real) group: matmuls -> bounce -> c -> tail
            wengs = [nc.sync, nc.scalar, nc.sync, nc.scalar]
            rengs = [nc.scalar, nc.sync, nc.scalar, nc.sync]
            for g in range(NG):
                gsl = slice(2 * g, 2 * g + 2)
                h_ps = hps.tile([CH, 2, HALO], FP32, tag="h")
                for j in range(2):
                    rt = 2 * g + j
                    nc.tensor.matmul(h_ps[:, j, :], wbig[:, rt, :],
                                     xbf[:, rt, :], start=True, stop=True)
                nc.vector.tensor_copy(hall[:, gsl, :], h_ps[:])
                hw_ = wengs[g].dma_start(hd[:, gsl, :], hall[:, gsl, :])

                # rbig[p, rt, k] = H_rt[p, p + k]; custom APs are invisible
                # to Tile's dependency tracker -> explicit edges.
                diag = bass.AP(
                    hd[:].tensor, 2 * g * HALO,
                    [[RT * HALO + 1, CH], [HALO, 2], [1, NK]],
                )
                d_r = rengs[g].dma_start(rbig[:, gsl, :], diag)
                add_dep_helper(d_r.ins, hw_.ins, reason="deskew reads hd")

                cs_ps = cps.tile([1, 2 * NK], FP32, tag="cs")
                mm = nc.tensor.matmul(
                    cs_ps[:], ones_col[:],
                    rbig[:, gsl, :].rearrange("p a b -> p (a b)"),
                    start=True, stop=True,
                )
                add_dep_helper(mm.ins, d_r.ins, reason="rbig ready")

                # r = c / c0 for the pair, then diff into diffs[:, g, :]
                rec2 = persist.tile([1, 2], FP32, tag="rec", bufs=2)
                nc.vector.reciprocal(rec2[:], cs_ps[0:1, 0:2 * NK:NK])
                rn = rnorm[0:1, 2 * g * NK:(2 * g + 2) * NK]
                nc.vector.tensor_tensor(
                    rn.rearrange("p (a b) -> p a b", a=2),
                    cs_ps[:].rearrange("p (a b) -> p a b", a=2),
                    rec2[:].unsqueeze(2).broadcast_to([1, 2, NK]),
                    mybir.AluOpType.mult,
                )
                nc.vector.tensor_sub(
                    diffs[:, g, :],
                    rnorm[0:1, 2 * g * NK:(2 * g + 1) * NK],
                    rnorm[0:1, (2 * g + 1) * NK:(2 * g + 2) * NK],
                )

            # --- sum_k |diff| per group, one scalar each -> out
            nc.vector.tensor_reduce(
                absum[:], diffs[:], mybir.AxisListType.X, mybir.AluOpType.add,
                apply_absolute_value=True,
            )
            nc.sync.dma_start(out[0:1, :], absum[:])

    nc.compile()
    return nc


_CACHE = {}


def _get_program():
    if "nc" not in _CACHE:
        _CACHE["nc"] = build_program()
    return _CACHE["nc"]


def make_in_maps(fake: np.ndarray, real: np.ndarray):
    fake = np.asarray(fake, dtype=np.float32).reshape(B, L)
    real = np.asarray(real, dtype=np.float32).reshape(B, L)
    in_maps = []
    for c in range(N_CORES):
        rows = slice(c * ROWS_PER_CORE, (c + 1) * ROWS_PER_CORE)
        xin = np.empty((RT, L), dtype=np.float32)
        xin[0::2] = fake[rows]
        xin[1::2] = real[rows]
        in_maps.append({"xin": np.ascontiguousarray(xin)})
    return in_maps


def run(in_maps, **kwargs):
    """Run the SPMD program; returns (loss, BassKernelResults)."""
    res = run_bass_kernel_spmd(
        _get_program(), in_maps, list(range(N_CORES)), **kwargs
    )
    total = np.float64(0.0)
    for c in range(N_CORES):
        total += np.asarray(res.results[c]["out"], dtype=np.float64).sum()
    return np.float32(total / (B * NK)), res


def kernel(fake: np.ndarray, real: np.ndarray) -> np.ndarray:
    loss, _ = run(make_in_maps(fake, real))
    return loss


# revision 10
# speedup vs baseline: 1.0561x; 1.0561x over previous
"""AutoCorrelationLoss Trainium2 kernel (8-core SPMD, data-parallel over batch).

Math: for each row x (length L=8192), with com = L - 128 = 8064 = 128*63:
  ac[k] = mean(x0c * (Y_k - mean(Y_k)))  where x0c = x[:com] - mean(x[:com])
Since sum(x0c) = 0, the mean(Y_k) term vanishes:
  com * ac[k] = c[k] = sum_j x0c[j] * x[j+k]
Decompose j = 63*t + p (t<128, p<63) and let X2[t, f] = x[63t + f] (f<191),
W = X2[:, :63] - mean(x[:com]).  Then with H = W.T @ X2  ([63, 191]):
  c[k] = sum_{p<63} H[p, p+k]   (a skew sum, k = 0..128)
which a diagonal-stride DMA through a DRAM bounce turns into a plain
column sum (ones matmul).  r[k] = c[k]/c[0];
loss = mean_{b,k} |r_fake - r_real|.

Per core: 4 batch rows x {fake, real} = 8 row-tensors, interleaved as 4
groups (fake_i, real_i) so each group's de-skew bounce + normalize + |diff|
tail pipelines against later groups' matmuls.  All matmuls in bf16
(1 cycle/row vs fp32's 4); fp32 PSUM accumulate keeps the final scalar
well inside the 2e-2 gate (measured ~5e-5).
"""

import sys

sys.path.insert(0, "/opt/trn_rl_repo")

import numpy as np

import concourse.bacc as bacc
import concourse.bass as bass
import concourse.mybir as mybir
import concourse.tile as tile
from concourse.bass_utils import run_bass_kernel_spmd
from concourse.tile_rust import add_dep_helper

B, L = 32, 8192
NCOEF = 128            # lags 0..128 -> 129 values
NK = NCOEF + 1         # 129
COM = L - NCOEF        # 8064 = 128 * 63
CH = 63                # chunk width (free dim of weights / H partition dim)
NT = COM // CH         # 128 contraction chunks -> full partition dim
HALO = CH + NCOEF      # 191
N_CORES = 8
ROWS_PER_CORE = B // N_CORES      # 4 batch rows per core
RT = 2 * ROWS_PER_CORE            # 8 row-tensors: [f0 r0 f1 r1 f2 r2 f3 r3]
NG = ROWS_PER_CORE                # 4 (fake, real) groups
HRT = RT // 2                     # 4 rows per load half

FP32 = mybir.dt.float32
BF16 = mybir.dt.bfloat16


def build_program():
    # num_devices=1: the program is pure data-parallel SPMD (no collectives,
    # no partition_id), and the single-device program skips the per-core
    # dispatch preamble.
    nc = bacc.Bacc(
        "TRN2",
        target_bir_lowering=False,
        debug=False,
        num_devices=1,
    )

    xin = nc.dram_tensor("xin", (RT, L), FP32, kind="ExternalInput")
    out = nc.dram_tensor("out", (1, NG), FP32, kind="ExternalOutput")

    with tile.TileContext(nc) as tc:
        with (
            tc.tile_pool(name="persist", bufs=1) as persist,
            tc.tile_pool(name="hdp", bufs=1, space=bass.MemorySpace.DRAM) as hdp,
            tc.tile_pool(name="hps", bufs=3, space=bass.MemorySpace.PSUM) as hps,
            tc.tile_pool(name="bps", bufs=2, space=bass.MemorySpace.PSUM) as bps,
            tc.tile_pool(name="cps", bufs=2, space=bass.MemorySpace.PSUM) as cps,
        ):
            ones_bf = persist.tile([NT, NT], BF16)      # partition-bcast weights
            nc.vector.memset(ones_bf[:], 1.0)
            ones_col = persist.tile([CH, 1], BF16)      # column-sum weights
            nc.vector.memset(ones_col[:], 1.0)

            xall = persist.tile([NT, RT, HALO], FP32)   # halo'd input
            xbf = persist.tile([NT, RT, HALO], BF16)    # bf16 rhs
            rowsums = persist.tile([NT, RT], FP32)
            msc = persist.tile([NT, RT], BF16)          # per-chunk means
            wbig = persist.tile([NT, RT, CH], BF16)     # centered weights
            hall = persist.tile([CH, RT, HALO], BF16)   # H matrices (SBUF)
            rbig = persist.tile([CH, RT, NK], BF16)     # de-skewed diagonals
            rnorm = persist.tile([1, RT * NK], FP32)    # c / c0
            diffs = persist.tile([1, NG, NK], FP32)     # r_fake - r_real
            absum = persist.tile([1, NG], FP32)

            hd = hdp.tile([CH, RT, HALO], BF16)         # DRAM de-skew bounce

            # --- loads (two halves, parallel trigger queues) + mean chain
            for h in range(2):
                sl = slice(h * HRT, (h + 1) * HRT)
                eng = nc.sync if h == 0 else nc.scalar
                src = bass.AP(xin, h * HRT * L, [[CH, NT], [L, HRT], [1, HALO]])
                eng.dma_start(xall[:, sl, :], src)

                # 1-input cast runs at line rate on GpSimd and frees DVE
                nc.gpsimd.tensor_copy(xbf[:, sl, :], xall[:, sl, :])
                nc.vector.tensor_reduce(
                    rowsums[:, sl], xall[:, sl, 0:CH],
                    mybir.AxisListType.X, mybir.AluOpType.add,
                )
                nc.vector.tensor_scalar_mul(msc[:, sl], rowsums[:, sl],
                                            1.0 / COM)
                # broadcast sum of per-chunk means (= row mean) over partitions
                mb = bps.tile([NT, HRT], FP32, tag="mb")
                nc.tensor.matmul(mb[:], ones_bf[:], msc[:, sl],
                                 start=True, stop=True)
                nc.vector.tensor_tensor(
                    wbig[:, sl, :], xall[:, sl, 0:CH],
                    mb[:].unsqueeze(2).broadcast_to([NT, HRT, CH]),
                    mybir.AluOpType.subtract,
                )

            # --- per (fake, real) group: matmuls -> bounce -> c -> tail
            # All hd writes on the sync HWDGE ring and all diag reads on the
            # scalar ring: each ring's FIFO then matches pipeline order, so
            # group g+1's write never queues behind group g's read.
            for g in range(NG):
                gsl = slice(2 * g, 2 * g + 2)
                h_ps = hps.tile([CH, 2, HALO], FP32, tag="h")
                for j in range(2):
                    rt = 2 * g + j
                    nc.tensor.matmul(h_ps[:, j, :], wbig[:, rt, :],
                                     xbf[:, rt, :], start=True, stop=True)
                nc.vector.tensor_copy(hall[:, gsl, :], h_ps[:])
                hw_ = nc.sync.dma_start(hd[:, gsl, :], hall[:, gsl, :])

                # rbig[p, rt, k] = H_rt[p, p + k]; custom APs are invisible
                # to Tile's dependency tracker -> explicit edges.
                diag = bass.AP(
                    hd[:].tensor, 2 * g * HALO,
                    [[RT * HALO + 1, CH], [HALO, 2], [1, NK]],
                )
                d_r = nc.scalar.dma_start(rbig[:, gsl, :], diag)
                add_dep_helper(d_r.ins, hw_.ins, reason="deskew reads hd")

                cs_ps = cps.tile([1, 2 * NK], FP32, tag="cs")
                mm = nc.tensor.matmul(
                    cs_ps[:], ones_col[:],
                    rbig[:, gsl, :].rearrange("p a b -> p (a b)"),
                    start=True, stop=True,
                )
                add_dep_helper(mm.ins, d_r.ins, reason="rbig ready")

                # r = c / c0 for the pair, then diff into diffs[:, g, :]
                rec2 = persist.tile([1, 2], FP32, tag="rec", bufs=2)
                nc.vector.reciprocal(rec2[:], cs_ps[0:1, 0:2 * NK:NK])
                rn = rnorm[0:1, 2 * g * NK:(2 * g + 2) * NK]
                nc.vector.tensor_tensor(
                    rn.rearrange("p (a b) -> p a b", a=2),
                    cs_ps[:].rearrange("p (a b) -> p a b", a=2),
                    rec2[:].unsqueeze(2).broadcast_to([1, 2, NK]),
                    mybir.AluOpType.mult,
                )
                nc.vector.tensor_sub(
                    diffs[:, g, :],
                    rnorm[0:1, 2 * g * NK:(2 * g + 1) * NK],
                    rnorm[0:1, (2 * g + 1) * NK:(2 * g + 2) * NK],
                )
                nc.vector.tensor_reduce(
                    absum[:, g:g + 1], diffs[:, g, :],
                    mybir.AxisListType.X, mybir.AluOpType.add,
                    apply_absolute_value=True,
                )

            nc.sync.dma_start(out[0:1, :], absum[:])

    nc.compile()
    return nc


_CACHE = {}


def _get_program():
    if "nc" not in _CACHE:
        _CACHE["nc"] = build_program()
    return _CACHE["nc"]


def make_in_maps(fake: np.ndarray, real: np.ndarray):
    fake = np.asarray(fake, dtype=np.float32).reshape(B, L)
    real = np.asarray(real, dtype=np.float32).reshape(B, L)
    in_maps = []
    for c in range(N_CORES):
        rows = slice(c * ROWS_PER_CORE, (c + 1) * ROWS_PER_CORE)
        xin = np.empty((RT, L), dtype=np.float32)
        xin[0::2] = fake[rows]
        xin[1::2] = real[rows]
        in_maps.append({"xin": np.ascontiguousarray(xin)})
    return in_maps


def run(in_maps, **kwargs):
    """Run the SPMD program; returns (loss, BassKernelResults)."""
    res = run_bass_kernel_spmd(
        _get_program(), in_maps, list(range(N_CORES)), **kwargs
    )
    total = np.float64(0.0)
    for c in range(N_CORES):
        total += np.asarray(res.results[c]["out"], dtype=np.float64).sum()
    return np.float32(total / (B * NK)), res


def kernel(fake: np.ndarray, real: np.ndarray) -> np.ndarray:
    loss, _ = run(make_in_maps(fake, real))
    return loss


# revision 12
# speedup vs baseline: 1.4511x; 1.3740x over previous
"""AutoCorrelationLoss Trainium2 kernel (8-core SPMD, data-parallel over batch).

Math: for each row x (length L=8192), with com = L - 128 = 8064 = 128*63:
  ac[k] = mean(x0c * (Y_k - mean(Y_k)))  where x0c = x[:com] - mean(x[:com])
Since sum(x0c) = 0, both the mean(Y_k) term and any constant shift of the
lagged windows vanish:
  com * ac[k] = c[k] = sum_j x0c[j] * (x[j+k] - m)
Decompose j = 63*t + p (t<128, p<63) and let XC[t, f] = x[63t + f] - m
(f<191, m = mean(x[:com])).  Then with H = XC[:, :63].T @ XC  ([63, 191]):
  c[k] = sum_{p<63} H[p, p+k]   (a skew sum, k = 0..128)
which a diagonal-stride DMA through a DRAM bounce turns into a plain
column sum (ones matmul).  r[k] = c[k]/c[0];
loss = mean_{b,k} |r_fake - r_real|.

Per core: 4 batch rows x {fake, real} = 8 row-tensors, interleaved as 4
groups (fake_i, real_i) so each group's de-skew bounce + normalize + |diff|
tail pipelines against later groups' matmuls.  Each group gets its own
DRAM bounce tile: the diagonal read uses a custom AP which Tile tracks
conservatively as whole-tensor, so a shared tile would serialize group
g+1's write behind group g's read.  All matmuls in bf16 (1 cycle/row vs
fp32's 4); fp32 PSUM accumulate keeps the final scalar well inside the
2e-2 gate (measured ~1e-4 per lag, ~1e-3 on the loss).
"""

import sys

sys.path.insert(0, "/opt/trn_rl_repo")

import numpy as np

import concourse.bacc as bacc
import concourse.bass as bass
import concourse.mybir as mybir
import concourse.tile as tile
from concourse.bass_utils import run_bass_kernel_spmd
from concourse.tile_rust import add_dep_helper

B, L = 32, 8192
NCOEF = 128            # lags 0..128 -> 129 values
NK = NCOEF + 1         # 129
COM = L - NCOEF        # 8064 = 128 * 63
CH = 63                # chunk width (free dim of weights / H partition dim)
NT = COM // CH         # 128 contraction chunks -> full partition dim
HALO = CH + NCOEF      # 191
N_CORES = 8
ROWS_PER_CORE = B // N_CORES      # 4 batch rows per core
RT = 2 * ROWS_PER_CORE            # 8 row-tensors: [f0 r0 f1 r1 f2 r2 f3 r3]
NG = ROWS_PER_CORE                # 4 (fake, real) groups
HRT = RT // 2                     # 4 rows per load half

FP32 = mybir.dt.float32
BF16 = mybir.dt.bfloat16


def build_program():
    nc = bacc.Bacc(
        "TRN2",
        target_bir_lowering=False,
        debug=False,
        num_devices=1,
    )

    xin = nc.dram_tensor("xin", (RT, L), FP32, kind="ExternalInput")
    out = nc.dram_tensor("out", (1, NG), FP32, kind="ExternalOutput")

    with tile.TileContext(nc) as tc:
        with (
            tc.tile_pool(name="persist", bufs=1) as persist,
            tc.tile_pool(name="hdp", bufs=1, space=bass.MemorySpace.DRAM) as hdp,
            tc.tile_pool(name="hps", bufs=3, space=bass.MemorySpace.PSUM) as hps,
            tc.tile_pool(name="bps", bufs=2, space=bass.MemorySpace.PSUM) as bps,
            tc.tile_pool(name="cps", bufs=2, space=bass.MemorySpace.PSUM) as cps,
        ):
            ones_bf = persist.tile([NT, NT], BF16)      # partition-bcast weights
            nc.vector.memset(ones_bf[:], 1.0)
            ones_col = persist.tile([CH, 1], BF16)      # column-sum weights
            nc.vector.memset(ones_col[:], 1.0)

            xall = persist.tile([NT, RT, HALO], FP32)   # halo'd input
            xc = persist.tile([NT, RT, HALO], BF16)     # centered bf16 operands
            rowsums = persist.tile([NT, RT], FP32)
            msc = persist.tile([NT, RT], BF16)          # per-chunk means
            hall = persist.tile([CH, RT, HALO], BF16)   # H matrices (SBUF)
            rbig = persist.tile([CH, RT, NK], BF16)     # de-skewed diagonals
            rnorm = persist.tile([1, RT * NK], FP32)    # c / c0
            diffs = persist.tile([1, NG, NK], FP32)     # r_fake - r_real
            absum = persist.tile([1, NG], FP32)

            hds = [hdp.tile([CH, 2, HALO], BF16, name=f"hd{g}")
                   for g in range(NG)]                  # per-group bounce

            # --- loads (two halves, parallel trigger queues) + mean chain
            for h in range(2):
                sl = slice(h * HRT, (h + 1) * HRT)
                eng = nc.sync if h == 0 else nc.scalar
                src = bass.AP(xin, h * HRT * L, [[CH, NT], [L, HRT], [1, HALO]])
                eng.dma_start(xall[:, sl, :], src)

                nc.vector.tensor_reduce(
                    rowsums[:, sl], xall[:, sl, 0:CH],
                    mybir.AxisListType.X, mybir.AluOpType.add,
                )
                nc.vector.tensor_scalar_mul(msc[:, sl], rowsums[:, sl],
                                            1.0 / COM)
                # broadcast sum of per-chunk means (= row mean) over partitions
                mb = bps.tile([NT, HRT], FP32, tag="mb")
                nc.tensor.matmul(mb[:], ones_bf[:], msc[:, sl],
                                 start=True, stop=True)
                # center + cast in one op; centering the lagged columns too is
                # free in exact math (sum(x0c) = 0)
                nc.vector.tensor_tensor(
                    xc[:, sl, :], xall[:, sl, :],
                    mb[:].unsqueeze(2).broadcast_to([NT, HRT, HALO]),
                    mybir.AluOpType.subtract,
                )

            # --- per (fake, real) group: matmuls -> bounce -> c -> tail.
            # All hd writes ride the sync HWDGE ring, all diag reads the
            # scalar ring, so each ring's FIFO matches pipeline order.
            for g in range(NG):
                gsl = slice(2 * g, 2 * g + 2)
                hd = hds[g]
                h_ps = hps.tile([CH, 2, HALO], FP32, tag="h")
                for j in range(2):
                    rt = 2 * g + j
                    nc.tensor.matmul(h_ps[:, j, :], xc[:, rt, 0:CH],
                                     xc[:, rt, :], start=True, stop=True)
                nc.vector.tensor_copy(hall[:, gsl, :], h_ps[:])
                hw_ = nc.sync.dma_start(hd[:], hall[:, gsl, :])

                # rbig[p, rt, k] = H_rt[p, p + k]; custom APs are invisible
                # to Tile's dependency tracker -> explicit edges.
                diag = bass.AP(hd[:].tensor, 0,
                               [[2 * HALO + 1, CH], [HALO, 2], [1, NK]])
                d_r = nc.scalar.dma_start(rbig[:, gsl, :], diag)
                add_dep_helper(d_r.ins, hw_.ins, reason="deskew reads hd")

                cs_ps = cps.tile([1, 2 * NK], FP32, tag="cs")
                mm = nc.tensor.matmul(
                    cs_ps[:], ones_col[:],
                    rbig[:, gsl, :].rearrange("p a b -> p (a b)"),
                    start=True, stop=True,
                )
                add_dep_helper(mm.ins, d_r.ins, reason="rbig ready")

                # r = c / c0 for the pair, then sum_k |r_f - r_r|
                rec2 = persist.tile([1, 2], FP32, tag="rec", bufs=2)
                nc.vector.reciprocal(rec2[:], cs_ps[0:1, 0:2 * NK:NK])
                rn = rnorm[0:1, 2 * g * NK:(2 * g + 2) * NK]
                nc.vector.tensor_tensor(
                    rn.rearrange("p (a b) -> p a b", a=2),
                    cs_ps[:].rearrange("p (a b) -> p a b", a=2),
                    rec2[:].unsqueeze(2).broadcast_to([1, 2, NK]),
                    mybir.AluOpType.mult,
                )
                nc.vector.tensor_sub(
                    diffs[:, g, :],
                    rnorm[0:1, 2 * g * NK:(2 * g + 1) * NK],
                    rnorm[0:1, (2 * g + 1) * NK:(2 * g + 2) * NK],
                )
                nc.vector.tensor_reduce(
                    absum[:, g:g + 1], diffs[:, g, :],
                    mybir.AxisListType.X, mybir.AluOpType.add,
                    apply_absolute_value=True,
                )

            nc.sync.dma_start(out[0:1, :], absum[:])

    nc.compile()
    return nc


_CACHE = {}


def _get_program():
    if "nc" not in _CACHE:
        _CACHE["nc"] = build_program()
    return _CACHE["nc"]


def make_in_maps(fake: np.ndarray, real: np.ndarray):
    fake = np.asarray(fake, dtype=np.float32).reshape(B, L)
    real = np.asarray(real, dtype=np.float32).reshape(B, L)
    in_maps = []
    for c in range(N_CORES):
        rows = slice(c * ROWS_PER_CORE, (c + 1) * ROWS_PER_CORE)
        xin = np.empty((RT, L), dtype=np.float32)
        xin[0::2] = fake[rows]
        xin[1::2] = real[rows]
        in_maps.append({"xin": np.ascontiguousarray(xin)})
    return in_maps


def run(in_maps, **kwargs):
    """Run the SPMD program; returns (loss, BassKernelResults)."""
    res = run_bass_kernel_spmd(
        _get_program(), in_maps, list(range(N_CORES)), **kwargs
    )
    total = np.float64(0.0)
    for c in range(N_CORES):
        total += np.asarray(res.results[c]["out"], dtype=np.float64).sum()
    return np.float32(total / (B * NK)), res


def kernel(fake: np.ndarray, real: np.ndarray) -> np.ndarray:
    loss, _ = run(make_in_maps(fake, real))
    return loss


# revision 13
# speedup vs baseline: 1.5020x; 1.0351x over previous
"""AutoCorrelationLoss Trainium2 kernel (8-core SPMD, data-parallel over batch).

Math: for each row x (length L=8192), with com = L - 128 = 8064 = 128*63:
  ac[k] = mean(x0c * (Y_k - mean(Y_k)))  where x0c = x[:com] - mean(x[:com])
Since sum(x0c) = 0, both the mean(Y_k) term and any constant shift of the
lagged windows vanish:
  com * ac[k] = c[k] = sum_j x0c[j] * (x[j+k] - m)
Decompose j = 63*t + p (t<128, p<63) and let XC[t, f] = x[63t + f] - m
(f<191, m = mean(x[:com])).  Then with H = XC[:, :63].T @ XC  ([63, 191]):
  c[k] = sum_{p<63} H[p, p+k]   (a skew sum, k = 0..128)
which a diagonal-stride DMA through a DRAM bounce turns into a plain
column sum (ones matmul).  r[k] = c[k]/c[0];
loss = mean_{b,k} |r_fake - r_real|.

Per core: 4 batch rows x {fake, real} = 8 row-tensors, interleaved as 4
groups (fake_i, real_i) so each group's de-skew bounce + normalize + |diff|
tail pipelines against later groups' matmuls.  Each group gets its own
DRAM bounce tile: the diagonal read uses a custom AP which Tile tracks
conservatively as whole-tensor, so a shared tile would serialize group
g+1's write behind group g's read.  All matmuls in bf16 (1 cycle/row vs
fp32's 4); fp32 PSUM accumulate keeps the final scalar well inside the
2e-2 gate (measured ~1e-4 per lag, ~1e-3 on the loss).
"""

import sys

sys.path.insert(0, "/opt/trn_rl_repo")

import numpy as np

import concourse.bacc as bacc
import concourse.bass as bass
import concourse.mybir as mybir
import concourse.tile as tile
from concourse.bass_utils import run_bass_kernel_spmd
from concourse.tile_rust import add_dep_helper

B, L = 32, 8192
NCOEF = 128            # lags 0..128 -> 129 values
NK = NCOEF + 1         # 129
COM = L - NCOEF        # 8064 = 128 * 63
CH = 63                # chunk width (free dim of weights / H partition dim)
NT = COM // CH         # 128 contraction chunks -> full partition dim
HALO = CH + NCOEF      # 191
N_CORES = 8
ROWS_PER_CORE = B // N_CORES      # 4 batch rows per core
RT = 2 * ROWS_PER_CORE            # 8 row-tensors: [f0 r0 f1 r1 f2 r2 f3 r3]
NG = ROWS_PER_CORE                # 4 (fake, real) groups
HRT = RT // 2                     # 4 rows per load half

FP32 = mybir.dt.float32
BF16 = mybir.dt.bfloat16


def build_program():
    nc = bacc.Bacc(
        "TRN2",
        target_bir_lowering=False,
        debug=False,
        num_devices=1,
    )

    xin = nc.dram_tensor("xin", (RT, L), FP32, kind="ExternalInput")
    out = nc.dram_tensor("out", (1, NG), FP32, kind="ExternalOutput")

    with tile.TileContext(nc) as tc:
        with (
            tc.tile_pool(name="persist", bufs=1) as persist,
            tc.tile_pool(name="hdp", bufs=1, space=bass.MemorySpace.DRAM) as hdp,
            tc.tile_pool(name="hps", bufs=3, space=bass.MemorySpace.PSUM) as hps,
            tc.tile_pool(name="bps", bufs=2, space=bass.MemorySpace.PSUM) as bps,
            tc.tile_pool(name="cps", bufs=2, space=bass.MemorySpace.PSUM) as cps,
        ):
            ones_bf = persist.tile([NT, NT], BF16)      # partition-bcast weights
            nc.vector.memset(ones_bf[:], 1.0)
            ones_col = persist.tile([CH, 1], BF16)      # column-sum weights
            nc.vector.memset(ones_col[:], 1.0)

            xall = persist.tile([NT, RT, HALO], FP32)   # halo'd input
            xc = persist.tile([NT, RT, HALO], BF16)     # centered bf16 operands
            rowsums = persist.tile([NT, RT], FP32)
            msc = persist.tile([NT, RT], BF16)          # per-chunk means
            hall = persist.tile([CH, RT, HALO], BF16)   # H matrices (SBUF)
            rbig = persist.tile([CH, RT, NK], BF16)     # de-skewed diagonals
            rnorm = persist.tile([1, RT * NK], FP32)    # c / c0
            diffs = persist.tile([1, NG, NK], FP32)     # r_fake - r_real
            absum = persist.tile([1, NG], FP32)

            hds = [hdp.tile([CH, 2, HALO], BF16, name=f"hd{g}")
                   for g in range(NG)]                  # per-group bounce

            # --- per (fake, real) group: load -> mean -> matmuls -> bounce
            # -> c -> tail, fully pipelined across the 4 groups.  Loads and
            # hd writes ride the sync HWDGE ring, diag reads (and two loads)
            # the scalar ring, so each ring's FIFO matches pipeline order.
            for g in range(NG):
                gsl = slice(2 * g, 2 * g + 2)
                eng = nc.sync if g % 2 == 0 else nc.scalar
                src = bass.AP(xin, 2 * g * L, [[CH, NT], [L, 2], [1, HALO]])
                eng.dma_start(xall[:, gsl, :], src)

                nc.vector.tensor_reduce(
                    rowsums[:, gsl], xall[:, gsl, 0:CH],
                    mybir.AxisListType.X, mybir.AluOpType.add,
                )
                nc.vector.tensor_scalar_mul(msc[:, gsl], rowsums[:, gsl],
                                            1.0 / COM)
                # broadcast sum of per-chunk means (= row mean) over partitions
                mb = bps.tile([NT, 2], FP32, tag="mb")
                nc.tensor.matmul(mb[:], ones_bf[:], msc[:, gsl],
                                 start=True, stop=True)
                # center + cast in one op; centering the lagged columns too is
                # free in exact math (sum(x0c) = 0)
                nc.vector.tensor_tensor(
                    xc[:, gsl, :], xall[:, gsl, :],
                    mb[:].unsqueeze(2).broadcast_to([NT, 2, HALO]),
                    mybir.AluOpType.subtract,
                )

                hd = hds[g]
                h_ps = hps.tile([CH, 2, HALO], FP32, tag="h")
                for j in range(2):
                    rt = 2 * g + j
                    nc.tensor.matmul(h_ps[:, j, :], xc[:, rt, 0:CH],
                                     xc[:, rt, :], start=True, stop=True)
                nc.vector.tensor_copy(hall[:, gsl, :], h_ps[:])
                hw_ = nc.sync.dma_start(hd[:], hall[:, gsl, :])

                # rbig[p, rt, k] = H_rt[p, p + k]; custom APs are invisible
                # to Tile's dependency tracker -> explicit edges.
                diag = bass.AP(hd[:].tensor, 0,
                               [[2 * HALO + 1, CH], [HALO, 2], [1, NK]])
                d_r = nc.scalar.dma_start(rbig[:, gsl, :], diag)
                add_dep_helper(d_r.ins, hw_.ins, reason="deskew reads hd")

                cs_ps = cps.tile([1, 2 * NK], FP32, tag="cs")
                mm = nc.tensor.matmul(
                    cs_ps[:], ones_col[:],
                    rbig[:, gsl, :].rearrange("p a b -> p (a b)"),
                    start=True, stop=True,
                )
                add_dep_helper(mm.ins, d_r.ins, reason="rbig ready")

                # r = c / c0 for the pair, then sum_k |r_f - r_r|
                rec2 = persist.tile([1, 2], FP32, tag="rec", bufs=2)
                nc.vector.reciprocal(rec2[:], cs_ps[0:1, 0:2 * NK:NK])
                rn = rnorm[0:1, 2 * g * NK:(2 * g + 2) * NK]
                nc.vector.tensor_tensor(
                    rn.rearrange("p (a b) -> p a b", a=2),
                    cs_ps[:].rearrange("p (a b) -> p a b", a=2),
                    rec2[:].unsqueeze(2).broadcast_to([1, 2, NK]),
                    mybir.AluOpType.mult,
                )
                nc.vector.tensor_sub(
                    diffs[:, g, :],
                    rnorm[0:1, 2 * g * NK:(2 * g + 1) * NK],
                    rnorm[0:1, (2 * g + 1) * NK:(2 * g + 2) * NK],
                )
                nc.vector.tensor_reduce(
                    absum[:, g:g + 1], diffs[:, g, :],
                    mybir.AxisListType.X, mybir.AluOpType.add,
                    apply_absolute_value=True,
                )

            nc.sync.dma_start(out[0:1, :], absum[:])

    nc.compile()
    return nc


_CACHE = {}


def _get_program():
    if "nc" not in _CACHE:
        _CACHE["nc"] = build_program()
    return _CACHE["nc"]


def make_in_maps(fake: np.ndarray, real: np.ndarray):
    fake = np.asarray(fake, dtype=np.float32).reshape(B, L)
    real = np.asarray(real, dtype=np.float32).reshape(B, L)
    in_maps = []
    for c in range(N_CORES):
        rows = slice(c * ROWS_PER_CORE, (c + 1) * ROWS_PER_CORE)
        xin = np.empty((RT, L), dtype=np.float32)
        xin[0::2] = fake[rows]
        xin[1::2] = real[rows]
        in_maps.append({"xin": np.ascontiguousarray(xin)})
    return in_maps


def run(in_maps, **kwargs):
    """Run the SPMD program; returns (loss, BassKernelResults)."""
    res = run_bass_kernel_spmd(
        _get_program(), in_maps, list(range(N_CORES)), **kwargs
    )
    total = np.float64(0.0)
    for c in range(N_CORES):
        total += np.asarray(res.results[c]["out"], dtype=np.float64).sum()
    return np.float32(total / (B * NK)), res


def kernel(fake: np.ndarray, real: np.ndarray) -> np.ndarray:
    loss, _ = run(make_in_maps(fake, real))
    return loss


# revision 14
# speedup vs baseline: 1.5294x; 1.0182x over previous
"""AutoCorrelationLoss Trainium2 kernel (8-core SPMD, data-parallel over batch).

Math: for each row x (length L=8192), with com = L - 128 = 8064 = 128*63:
  ac[k] = mean(x0c * (Y_k - mean(Y_k)))  where x0c = x[:com] - mean(x[:com])
Since sum(x0c) = 0, both the mean(Y_k) term and any constant shift of the
lagged windows vanish:
  com * ac[k] = c[k] = sum_j x0c[j] * (x[j+k] - m)
Decompose j = 63*t + p (t<128, p<63) and let XC[t, f] = x[63t + f] - m
(f<191, m = mean(x[:com])).  Then with H = XC[:, :63].T @ XC  ([63, 191]):
  c[k] = sum_{p<63} H[p, p+k]   (a skew sum, k = 0..128)
which a diagonal-stride DMA through a DRAM bounce turns into a plain
column sum (ones matmul).  r[k] = c[k]/c[0];
loss = mean_{b,k} |r_fake - r_real|.

Per core: 4 batch rows x {fake, real} = 8 row-tensors, interleaved as 4
groups (fake_i, real_i) so each group's de-skew bounce + normalize + |diff|
tail pipelines against later groups' matmuls.  Each group gets its own
DRAM bounce tile: the diagonal read uses a custom AP which Tile tracks
conservatively as whole-tensor, so a shared tile would serialize group
g+1's write behind group g's read.  All matmuls in bf16 (1 cycle/row vs
fp32's 4); fp32 PSUM accumulate keeps the final scalar well inside the
2e-2 gate (measured ~1e-4 per lag, ~1e-3 on the loss).
"""

import sys

sys.path.insert(0, "/opt/trn_rl_repo")

import numpy as np

import concourse.bacc as bacc
import concourse.bass as bass
import concourse.mybir as mybir
import concourse.tile as tile
from concourse.bass_utils import run_bass_kernel_spmd
from concourse.tile_rust import add_dep_helper

B, L = 32, 8192
NCOEF = 128            # lags 0..128 -> 129 values
NK = NCOEF + 1         # 129
COM = L - NCOEF        # 8064 = 128 * 63
CH = 63                # chunk width (free dim of weights / H partition dim)
NT = COM // CH         # 128 contraction chunks -> full partition dim
HALO = CH + NCOEF      # 191
N_CORES = 8
ROWS_PER_CORE = B // N_CORES      # 4 batch rows per core
RT = 2 * ROWS_PER_CORE            # 8 row-tensors: [f0 r0 f1 r1 f2 r2 f3 r3]
NG = ROWS_PER_CORE                # 4 (fake, real) groups
HRT = RT // 2                     # 4 rows per load half

FP32 = mybir.dt.float32
BF16 = mybir.dt.bfloat16


def build_program():
    nc = bacc.Bacc(
        "TRN2",
        target_bir_lowering=False,
        debug=False,
        num_devices=1,
    )

    xin = nc.dram_tensor("xin", (RT, L), FP32, kind="ExternalInput")
    out = nc.dram_tensor("out", (1, NG), FP32, kind="ExternalOutput")

    with tile.TileContext(nc) as tc:
        with (
            tc.tile_pool(name="persist", bufs=1) as persist,
            tc.tile_pool(name="hdp", bufs=1, space=bass.MemorySpace.DRAM) as hdp,
            tc.tile_pool(name="hps", bufs=3, space=bass.MemorySpace.PSUM) as hps,
            tc.tile_pool(name="bps", bufs=2, space=bass.MemorySpace.PSUM) as bps,
            tc.tile_pool(name="cps", bufs=2, space=bass.MemorySpace.PSUM) as cps,
        ):
            ones_bf = persist.tile([NT, NT], BF16)      # partition-bcast weights
            nc.vector.memset(ones_bf[:], 1.0)
            ones_col = persist.tile([CH, 1], BF16)      # column-sum weights
            nc.vector.memset(ones_col[:], 1.0)

            xall = persist.tile([NT, RT, HALO], FP32)   # halo'd input
            xc = persist.tile([NT, RT, HALO], BF16)     # centered bf16 operands
            rowsums = persist.tile([NT, RT], FP32)
            msc = persist.tile([NT, RT], BF16)          # per-chunk means
            hall = persist.tile([CH, RT, HALO], BF16)   # H matrices (SBUF)
            rbig = persist.tile([CH, RT, NK], BF16)     # de-skewed diagonals
            rnorm = persist.tile([1, RT * NK], FP32)    # c / c0
            diffs = persist.tile([1, NG, NK], FP32)     # r_fake - r_real
            absum = persist.tile([1, NG], FP32)

            hds = [hdp.tile([CH, 2, HALO], BF16, name=f"hd{g}")
                   for g in range(NG)]                  # per-group bounce

            # --- per (fake, real) group: load -> mean -> matmuls -> bounce
            # -> c -> tail, fully pipelined across the 4 groups.  Loads and
            # hd writes ride the sync HWDGE ring, diag reads (and two loads)
            # the scalar ring, so each ring's FIFO matches pipeline order.
            # loads spread over three DGE paths (sync/scalar HWDGE + gpsimd
            # SWDGE) so at most two transfers share a ring
            load_engs = [nc.sync, nc.scalar, nc.gpsimd, nc.sync]
            for g in range(NG):
                gsl = slice(2 * g, 2 * g + 2)
                src = bass.AP(xin, 2 * g * L, [[CH, NT], [L, 2], [1, HALO]])
                load_engs[g].dma_start(xall[:, gsl, :], src)

                nc.vector.tensor_reduce(
                    rowsums[:, gsl], xall[:, gsl, 0:CH],
                    mybir.AxisListType.X, mybir.AluOpType.add,
                )
                nc.vector.tensor_scalar_mul(msc[:, gsl], rowsums[:, gsl],
                                            1.0 / COM)
                # broadcast sum of per-chunk means (= row mean) over partitions
                mb = bps.tile([NT, 2], FP32, tag="mb")
                nc.tensor.matmul(mb[:], ones_bf[:], msc[:, gsl],
                                 start=True, stop=True)
                # center + cast in one op; centering the lagged columns too is
                # free in exact math (sum(x0c) = 0)
                nc.vector.tensor_tensor(
                    xc[:, gsl, :], xall[:, gsl, :],
                    mb[:].unsqueeze(2).broadcast_to([NT, 2, HALO]),
                    mybir.AluOpType.subtract,
                )

                hd = hds[g]
                h_ps = hps.tile([CH, 2, HALO], FP32, tag="h")
                for j in range(2):
                    rt = 2 * g + j
                    nc.tensor.matmul(h_ps[:, j, :], xc[:, rt, 0:CH],
                                     xc[:, rt, :], start=True, stop=True)
                nc.vector.tensor_copy(hall[:, gsl, :], h_ps[:])
                hw_ = nc.sync.dma_start(hd[:], hall[:, gsl, :])

                # rbig[p, rt, k] = H_rt[p, p + k]; custom APs are invisible
                # to Tile's dependency tracker -> explicit edges.
                diag = bass.AP(hd[:].tensor, 0,
                               [[2 * HALO + 1, CH], [HALO, 2], [1, NK]])
                d_r = nc.scalar.dma_start(rbig[:, gsl, :], diag)
                add_dep_helper(d_r.ins, hw_.ins, reason="deskew reads hd")

                cs_ps = cps.tile([1, 2 * NK], FP32, tag="cs")
                mm = nc.tensor.matmul(
                    cs_ps[:], ones_col[:],
                    rbig[:, gsl, :].rearrange("p a b -> p (a b)"),
                    start=True, stop=True,
                )
                add_dep_helper(mm.ins, d_r.ins, reason="rbig ready")

                # r = c / c0 for the pair, then sum_k |r_f - r_r|
                rec2 = persist.tile([1, 2], FP32, tag="rec", bufs=2)
                nc.vector.reciprocal(rec2[:], cs_ps[0:1, 0:2 * NK:NK])
                rn = rnorm[0:1, 2 * g * NK:(2 * g + 2) * NK]
                nc.vector.tensor_tensor(
                    rn.rearrange("p (a b) -> p a b", a=2),
                    cs_ps[:].rearrange("p (a b) -> p a b", a=2),
                    rec2[:].unsqueeze(2).broadcast_to([1, 2, NK]),
                    mybir.AluOpType.mult,
                )
                nc.vector.tensor_sub(
                    diffs[:, g, :],
                    rnorm[0:1, 2 * g * NK:(2 * g + 1) * NK],
                    rnorm[0:1, (2 * g + 1) * NK:(2 * g + 2) * NK],
                )
                nc.vector.tensor_reduce(
                    absum[:, g:g + 1], diffs[:, g, :],
                    mybir.AxisListType.X, mybir.AluOpType.add,
                    apply_absolute_value=True,
                )

            nc.sync.dma_start(out[0:1, :], absum[:])

    nc.compile()
    return nc


_CACHE = {}


def _get_program():
    if "nc" not in _CACHE:
        _CACHE["nc"] = build_program()
    return _CACHE["nc"]


def make_in_maps(fake: np.ndarray, real: np.ndarray):
    fake = np.asarray(fake, dtype=np.float32).reshape(B, L)
    real = np.asarray(real, dtype=np.float32).reshape(B, L)
    in_maps = []
    for c in range(N_CORES):
        rows = slice(c * ROWS_PER_CORE, (c + 1) * ROWS_PER_CORE)
        xin = np.empty((RT, L), dtype=np.float32)
        xin[0::2] = fake[rows]
        xin[1::2] = real[rows]
        in_maps.append({"xin": np.ascontiguousarray(xin)})
    return in_maps


def run(in_maps, **kwargs):
    """Run the SPMD program; returns (loss, BassKernelResults)."""
    res = run_bass_kernel_spmd(
        _get_program(), in_maps, list(range(N_CORES)), **kwargs
    )
    total = np.float64(0.0)
    for c in range(N_CORES):
        total += np.asarray(res.results[c]["out"], dtype=np.float64).sum()
    return np.float32(total / (B * NK)), res


def kernel(fake: np.ndarray, real: np.ndarray) -> np.ndarray:
    loss, _ = run(make_in_maps(fake, real))
    return loss


# revision 17
# speedup vs baseline: 1.5385x; 1.0060x over previous
"""AutoCorrelationLoss Trainium2 kernel (8-core SPMD, data-parallel over batch).

Math: for each row x (length L=8192), with com = L - 128 = 8064 = 128*63:
  ac[k] = mean(x0c * (Y_k - mean(Y_k)))  where x0c = x[:com] - mean(x[:com])
Since sum(x0c) = 0, both the mean(Y_k) term and any constant shift of the
lagged windows vanish:
  com * ac[k] = c[k] = sum_j x0c[j] * (x[j+k] - m)
Decompose j = 63*t + p (t<128, p<63) and let XC[t, f] = x[63t + f] - m
(f<191, m = mean(x[:com])).  Then with H = XC[:, :63].T @ XC  ([63, 191]):
  c[k] = sum_{p<63} H[p, p+k]   (a skew sum, k = 0..128)
which a diagonal-stride DMA through a DRAM bounce turns into a plain
column sum (ones matmul).  r[k] = c[k]/c[0];
loss = mean_{b,k} |r_fake - r_real|.

Per core: 4 batch rows x {fake, real} = 8 row-tensors, interleaved as 4
groups (fake_i, real_i) so each group's de-skew bounce + normalize + |diff|
tail pipelines against later groups' matmuls.  Each group gets its own
DRAM bounce tile: the diagonal read uses a custom AP which Tile tracks
conservatively as whole-tensor, so a shared tile would serialize group
g+1's write behind group g's read.  All matmuls in bf16 (1 cycle/row vs
fp32's 4); fp32 PSUM accumulate keeps the final scalar well inside the
2e-2 gate (measured ~1e-4 per lag, ~1e-3 on the loss).
"""

import sys

sys.path.insert(0, "/opt/trn_rl_repo")

import numpy as np

import concourse.bacc as bacc
import concourse.bass as bass
import concourse.mybir as mybir
import concourse.tile as tile
from concourse.bass_utils import run_bass_kernel_spmd
from concourse.tile_rust import add_dep_helper

B, L = 32, 8192
NCOEF = 128            # lags 0..128 -> 129 values
NK = NCOEF + 1         # 129
COM = L - NCOEF        # 8064 = 128 * 63
CH = 63                # chunk width (free dim of weights / H partition dim)
NT = COM // CH         # 128 contraction chunks -> full partition dim
HALO = CH + NCOEF      # 191
N_CORES = 8
ROWS_PER_CORE = B // N_CORES      # 4 batch rows per core
RT = 2 * ROWS_PER_CORE            # 8 row-tensors: [f0 r0 f1 r1 f2 r2 f3 r3]
NG = ROWS_PER_CORE                # 4 (fake, real) groups
HRT = RT // 2                     # 4 rows per load half

FP32 = mybir.dt.float32
BF16 = mybir.dt.bfloat16


def build_program():
    nc = bacc.Bacc(
        "TRN2",
        target_bir_lowering=False,
        debug=False,
        num_devices=1,
    )

    xin = nc.dram_tensor("xin", (RT, L), FP32, kind="ExternalInput")
    out = nc.dram_tensor("out", (1, NG), FP32, kind="ExternalOutput")

    with tile.TileContext(nc) as tc:
        with (
            tc.tile_pool(name="persist", bufs=1) as persist,
            tc.tile_pool(name="hdp", bufs=1, space=bass.MemorySpace.DRAM) as hdp,
            tc.tile_pool(name="hps", bufs=3, space=bass.MemorySpace.PSUM) as hps,
            tc.tile_pool(name="bps", bufs=2, space=bass.MemorySpace.PSUM) as bps,
            tc.tile_pool(name="cps", bufs=2, space=bass.MemorySpace.PSUM) as cps,
        ):
            ones_bf = persist.tile([NT, NT], BF16)      # partition-bcast weights
            nc.vector.memset(ones_bf[:], 1.0)
            ones_col = persist.tile([CH, 1], BF16)      # column-sum weights
            nc.vector.memset(ones_col[:], 1.0)

            xall = persist.tile([NT, RT, HALO], FP32)   # halo'd input
            xc = persist.tile([NT, RT, HALO], BF16)     # centered bf16 operands
            rowsums = persist.tile([NT, RT], FP32)
            msc = persist.tile([NT, RT], BF16)          # per-chunk means
            hall = persist.tile([CH, RT, HALO], BF16)   # H matrices (SBUF)
            rbig = persist.tile([CH, RT, NK], BF16)     # de-skewed diagonals
            rnorm = persist.tile([1, NG * NK], FP32)    # c_r / c0_r per group
            diffs = persist.tile([1, NG, NK], FP32)     # r_fake - r_real
            absum = persist.tile([1, NG], FP32)

            hds = [hdp.tile([CH, 2, HALO], BF16, name=f"hd{g}")
                   for g in range(NG)]                  # per-group bounce

            # --- per (fake, real) group: load -> mean -> matmuls -> bounce
            # -> c -> tail, fully pipelined across the 4 groups.  Loads and
            # hd writes ride the sync HWDGE ring, diag reads (and two loads)
            # the scalar ring, so each ring's FIFO matches pipeline order.
            # Loads are descriptor-generation bound (~8.5ns/descriptor,
            # serial per DGE path), so issue one DMA per row-tensor spread
            # over all three DGE paths (sync/scalar HWDGE + gpsimd SWDGE),
            # slotted so group completion order matches emission order.
            load_engs = [nc.sync, nc.scalar, nc.gpsimd, nc.sync,
                         nc.scalar, nc.gpsimd, nc.sync, nc.scalar]
            for g in range(NG):
                gsl = slice(2 * g, 2 * g + 2)
                for j in range(2):
                    rt = 2 * g + j
                    src = bass.AP(xin, rt * L, [[CH, NT], [1, HALO]])
                    load_engs[rt].dma_start(xall[:, rt, :], src)

                nc.vector.tensor_reduce(
                    rowsums[:, gsl], xall[:, gsl, 0:CH],
                    mybir.AxisListType.X, mybir.AluOpType.add,
                )
                nc.vector.tensor_scalar_mul(msc[:, gsl], rowsums[:, gsl],
                                            1.0 / COM)
                # broadcast sum of per-chunk means (= row mean) over partitions
                mb = bps.tile([NT, 2], FP32, tag="mb")
                nc.tensor.matmul(mb[:], ones_bf[:], msc[:, gsl],
                                 start=True, stop=True)
                # center + cast in one op; centering the lagged columns too is
                # free in exact math (sum(x0c) = 0)
                nc.vector.tensor_tensor(
                    xc[:, gsl, :], xall[:, gsl, :],
                    mb[:].unsqueeze(2).broadcast_to([NT, 2, HALO]),
                    mybir.AluOpType.subtract,
                )

                hd = hds[g]
                h_ps = hps.tile([CH, 2, HALO], FP32, tag="h")
                for j in range(2):
                    rt = 2 * g + j
                    nc.tensor.matmul(h_ps[:, j, :], xc[:, rt, 0:CH],
                                     xc[:, rt, :], start=True, stop=True)
                nc.vector.tensor_copy(hall[:, gsl, :], h_ps[:])
                hw_ = nc.sync.dma_start(hd[:], hall[:, gsl, :])

                # rbig[p, rt, k] = H_rt[p, p + k]; custom APs are invisible
                # to Tile's dependency tracker -> explicit edges.
                diag = bass.AP(hd[:].tensor, 0,
                               [[2 * HALO + 1, CH], [HALO, 2], [1, NK]])
                d_r = nc.scalar.dma_start(rbig[:, gsl, :], diag)
                add_dep_helper(d_r.ins, hw_.ins, reason="deskew reads hd")

                cs_ps = cps.tile([1, 2 * NK], FP32, tag="cs")
                mm = nc.tensor.matmul(
                    cs_ps[:], ones_col[:],
                    rbig[:, gsl, :].rearrange("p a b -> p (a b)"),
                    start=True, stop=True,
                )
                add_dep_helper(mm.ins, d_r.ins, reason="rbig ready")

                # diff = c_f/c0_f - c_r/c0_r, then sum_k |diff|
                rec2 = persist.tile([1, 2], FP32, tag="rec", bufs=2)
                nc.vector.reciprocal(rec2[:], cs_ps[0:1, 0:2 * NK:NK])
                rn = rnorm[0:1, g * NK:(g + 1) * NK]
                nc.vector.tensor_scalar_mul(rn, cs_ps[0:1, NK:2 * NK],
                                            rec2[0:1, 1:2])
                nc.vector.scalar_tensor_tensor(
                    diffs[:, g, :], cs_ps[0:1, 0:NK], rec2[0:1, 0:1], rn,
                    mybir.AluOpType.mult, mybir.AluOpType.subtract,
                )
                nc.vector.tensor_reduce(
                    absum[:, g:g + 1], diffs[:, g, :],
                    mybir.AxisListType.X, mybir.AluOpType.add,
                    apply_absolute_value=True,
                )

            nc.sync.dma_start(out[0:1, :], absum[:])

    nc.compile()
    return nc


_CACHE = {}


def _get_program():
    if "nc" not in _CACHE:
        _CACHE["nc"] = build_program()
    return _CACHE["nc"]


def make_in_maps(fake: np.ndarray, real: np.ndarray):
    fake = np.asarray(fake, dtype=np.float32).reshape(B, L)
    real = np.asarray(real, dtype=np.float32).reshape(B, L)
    in_maps = []
    for c in range(N_CORES):
        rows = slice(c * ROWS_PER_CORE, (c + 1) * ROWS_PER_CORE)
        xin = np.empty((RT, L), dtype=np.float32)
        xin[0::2] = fake[rows]
        xin[1::2] = real[rows]
        in_maps.append({"xin": np.ascontiguousarray(xin)})
    return in_maps


def run(in_maps, **kwargs):
    """Run the SPMD program; returns (loss, BassKernelResults)."""
    res = run_bass_kernel_spmd(
        _get_program(), in_maps, list(range(N_CORES)), **kwargs
    )
    total = np.float64(0.0)
    for c in range(N_CORES):
        total += np.asarray(res.results[c]["out"], dtype=np.float64).sum()
    return np.float32(total / (B * NK)), res


def kernel(fake: np.ndarray, real: np.ndarray) -> np.ndarray:
    loss, _ = run(make_in_maps(fake, real))
    return loss


# revision 22
# speedup vs baseline: 1.5826x; 1.0286x over previous
"""AutoCorrelationLoss Trainium2 kernel (8-core SPMD, data-parallel over batch).

Math: for each row x (length L=8192), with com = L - 128 = 8064 = 128*63:
  ac[k] = mean(x0c * (Y_k - mean(Y_k)))  where x0c = x[:com] - mean(x[:com])
Since sum(x0c) = 0, both the mean(Y_k) term and any constant shift of the
lagged windows vanish:
  com * ac[k] = c[k] = sum_j x0c[j] * (x[j+k] - m)
Decompose j = 63*t + p (t<128, p<63) and let XC[t, f] = x[63t + f] - m
(f<191, m = mean(x[:com])).  Then with H = XC[:, :63].T @ XC  ([63, 191]):
  c[k] = sum_{p<63} H[p, p+k]   (a skew sum, k = 0..128)
which a diagonal-stride DMA through a DRAM bounce turns into a plain
column sum (ones matmul).  r[k] = c[k]/c[0];
loss = mean_{b,k} |r_fake - r_real|.

Per core: 4 batch rows x {fake, real} = 8 row-tensors, interleaved as 4
groups (fake_i, real_i) so each group's de-skew bounce + normalize + |diff|
tail pipelines against later groups' matmuls.  Each group gets its own
DRAM bounce tile: the diagonal read uses a custom AP which Tile tracks
conservatively as whole-tensor, so a shared tile would serialize group
g+1's write behind group g's read.  All matmuls in bf16 (1 cycle/row vs
fp32's 4); fp32 PSUM accumulate keeps the final scalar well inside the
2e-2 gate (measured ~1e-4 per lag, ~1e-3 on the loss).
"""

import sys

sys.path.insert(0, "/opt/trn_rl_repo")

import numpy as np

import concourse.bacc as bacc
import concourse.bass as bass
import concourse.mybir as mybir
import concourse.tile as tile
from concourse.bass_utils import run_bass_kernel_spmd
from concourse.tile_rust import add_dep_helper

B, L = 32, 8192
NCOEF = 128            # lags 0..128 -> 129 values
NK = NCOEF + 1         # 129
COM = L - NCOEF        # 8064 = 128 * 63
CH = 63                # chunk width (free dim of weights / H partition dim)
NT = COM // CH         # 128 contraction chunks -> full partition dim
HALO = CH + NCOEF      # 191
N_CORES = 8
ROWS_PER_CORE = B // N_CORES      # 4 batch rows per core
RT = 2 * ROWS_PER_CORE            # 8 row-tensors: [f0 r0 f1 r1 f2 r2 f3 r3]
NG = ROWS_PER_CORE                # 4 (fake, real) groups
HRT = RT // 2                     # 4 rows per load half

FP32 = mybir.dt.float32
BF16 = mybir.dt.bfloat16


def build_program():
    nc = bacc.Bacc(
        "TRN2",
        target_bir_lowering=False,
        debug=False,
        num_devices=1,
    )

    xin = nc.dram_tensor("xin", (RT, L), FP32, kind="ExternalInput")
    out = nc.dram_tensor("out", (1, NG), FP32, kind="ExternalOutput")

    with tile.TileContext(nc) as tc:
        with (
            tc.tile_pool(name="persist", bufs=1) as persist,
            tc.tile_pool(name="hdp", bufs=1, space=bass.MemorySpace.DRAM) as hdp,
            tc.tile_pool(name="hps", bufs=3, space=bass.MemorySpace.PSUM) as hps,
            tc.tile_pool(name="bps", bufs=2, space=bass.MemorySpace.PSUM) as bps,
            tc.tile_pool(name="cps", bufs=2, space=bass.MemorySpace.PSUM) as cps,
        ):
            ones_bf = persist.tile([NT, NT], BF16)      # partition-bcast weights
            nc.vector.memset(ones_bf[:], 1.0)
            ones_col = persist.tile([CH, 1], BF16)      # column-sum weights
            nc.vector.memset(ones_col[:], 1.0)

            # prewarm the ACT function table (1.3us load) during the input
            # DMAs so the first centering op doesn't pay it
            warm = persist.tile([1, 1], FP32)
            nc.vector.memset(warm[:], 0.0)
            nc.scalar.activation(warm[:], warm[:],
                                 mybir.ActivationFunctionType.Identity)

            xall = persist.tile([NT, RT, HALO], FP32)   # halo'd input
            xc = persist.tile([NT, RT, HALO], BF16)     # centered bf16 operands
            rowsums = persist.tile([NT, RT], FP32)
            msc = persist.tile([NT, RT], BF16)          # per-chunk means
            hall = persist.tile([CH, RT, HALO], BF16)   # H matrices (SBUF)
            rbig = persist.tile([CH, RT, NK], BF16)     # de-skewed diagonals
            rnorm = persist.tile([1, NG * NK], FP32)    # c_r / c0_r per group
            diffs = persist.tile([1, NG, NK], FP32)     # r_fake - r_real
            absum = persist.tile([1, NG], FP32)

            hds = [hdp.tile([CH, 2, HALO], BF16, name=f"hd{g}")
                   for g in range(NG)]                  # per-group bounce

            # --- per (fake, real) group: load -> mean -> matmuls -> bounce
            # -> c -> tail, fully pipelined across the 4 groups.  Loads and
            # hd writes ride the sync HWDGE ring, diag reads (and two loads)
            # the scalar ring, so each ring's FIFO matches pipeline order.
            # Loads are descriptor-generation bound (~8.5ns/descriptor,
            # serial per DGE path), so issue one DMA per row-tensor spread
            # over all three DGE paths (sync/scalar HWDGE + gpsimd SWDGE),
            # slotted so group completion order matches emission order.
            load_engs = [nc.sync, nc.scalar, nc.gpsimd, nc.sync,
                         nc.scalar, nc.gpsimd, nc.sync, nc.scalar]
            for g in range(NG):
                gsl = slice(2 * g, 2 * g + 2)
                for j in range(2):
                    rt = 2 * g + j
                    src = bass.AP(xin, rt * L, [[CH, NT], [1, HALO]])
                    load_engs[rt].dma_start(xall[:, rt, :], src)

                nc.vector.tensor_reduce(
                    rowsums[:, gsl], xall[:, gsl, 0:CH],
                    mybir.AxisListType.X, mybir.AluOpType.add,
                )
                # negated scale: the broadcast mb is then -mean, usable as an
                # ACT bias directly
                nc.gpsimd.tensor_scalar_mul(msc[:, gsl], rowsums[:, gsl],
                                            -1.0 / COM)
                # broadcast sum of per-chunk means (= row mean) over partitions
                mb = bps.tile([NT, 2], FP32, tag="mb")
                nc.tensor.matmul(mb[:], ones_bf[:], msc[:, gsl],
                                 start=True, stop=True)
                # ACT bias APs must live in SBUF
                mbs = persist.tile([NT, 2], FP32, tag="mbs", bufs=2)
                nc.vector.tensor_copy(mbs[:], mb[:])
                # center + cast on the ACT engine: out = x + (-m); keeps the
                # DVE queue clear.  Centering the lagged columns too is free
                # in exact math (sum(x0c) = 0).
                for j in range(2):
                    rt = 2 * g + j
                    nc.scalar.activation(
                        xc[:, rt, :], xall[:, rt, :],
                        mybir.ActivationFunctionType.Identity,
                        bias=mbs[:, j:j + 1],
                    )

                hd = hds[g]
                h_ps = hps.tile([CH, 2, HALO], FP32, tag="h")
                for j in range(2):
                    rt = 2 * g + j
                    nc.tensor.matmul(h_ps[:, j, :], xc[:, rt, 0:CH],
                                     xc[:, rt, :], start=True, stop=True)
                nc.vector.tensor_copy(hall[:, gsl, :], h_ps[:])
                hw_ = nc.sync.dma_start(hd[:], hall[:, gsl, :])

                # rbig[p, rt, k] = H_rt[p, p + k]; custom APs are invisible
                # to Tile's dependency tracker -> explicit edges.
                diag = bass.AP(hd[:].tensor, 0,
                               [[2 * HALO + 1, CH], [HALO, 2], [1, NK]])
                d_r = nc.scalar.dma_start(rbig[:, gsl, :], diag)
                add_dep_helper(d_r.ins, hw_.ins, reason="deskew reads hd")

                cs_ps = cps.tile([1, 2 * NK], FP32, tag="cs")
                mm = nc.tensor.matmul(
                    cs_ps[:], ones_col[:],
                    rbig[:, gsl, :].rearrange("p a b -> p (a b)"),
                    start=True, stop=True,
                )
                add_dep_helper(mm.ins, d_r.ins, reason="rbig ready")

                # diff = c_f/c0_f - c_r/c0_r, then sum_k |diff|
                rec2 = persist.tile([1, 2], FP32, tag="rec", bufs=2)
                nc.vector.reciprocal(rec2[:], cs_ps[0:1, 0:2 * NK:NK])
                rn = rnorm[0:1, g * NK:(g + 1) * NK]
                nc.vector.tensor_scalar_mul(rn, cs_ps[0:1, NK:2 * NK],
                                            rec2[0:1, 1:2])
                nc.vector.scalar_tensor_tensor(
                    diffs[:, g, :], cs_ps[0:1, 0:NK], rec2[0:1, 0:1], rn,
                    mybir.AluOpType.mult, mybir.AluOpType.subtract,
                )
                nc.vector.tensor_reduce(
                    absum[:, g:g + 1], diffs[:, g, :],
                    mybir.AxisListType.X, mybir.AluOpType.add,
                    apply_absolute_value=True,
                )

            nc.sync.dma_start(out[0:1, :], absum[:])

    nc.compile()
    return nc


_CACHE = {}


def _get_program():
    if "nc" not in _CACHE:
        _CACHE["nc"] = build_program()
    return _CACHE["nc"]


def make_in_maps(fake: np.ndarray, real: np.ndarray):
    fake = np.asarray(fake, dtype=np.float32).reshape(B, L)
    real = np.asarray(real, dtype=np.float32).reshape(B, L)
    in_maps = []
    for c in range(N_CORES):
        rows = slice(c * ROWS_PER_CORE, (c + 1) * ROWS_PER_CORE)
        xin = np.empty((RT, L), dtype=np.float32)
        xin[0::2] = fake[rows]
        xin[1::2] = real[rows]
        in_maps.append({"xin": np.ascontiguousarray(xin)})
    return in_maps


def run(in_maps, **kwargs):
    """Run the SPMD program; returns (loss, BassKernelResults)."""
    res = run_bass_kernel_spmd(
        _get_program(), in_maps, list(range(N_CORES)), **kwargs
    )
    total = np.float64(0.0)
    for c in range(N_CORES):
        total += np.asarray(res.results[c]["out"], dtype=np.float64).sum()
    return np.float32(total / (B * NK)), res


def kernel(fake: np.ndarray, real: np.ndarray) -> np.ndarray:
    loss, _ = run(make_in_maps(fake, real))
    return loss
